# revision 33
# baseline (speedup 1.0000x reference)
"""Viterbi CRF decode kernel for Trainium2, data-parallel over batch on 8 cores.

Device computes the forward Viterbi max-plus scan (the O(S*B*T^2) bulk):
  part_t[b, j] = max_i((feats[t,b,j] + trans[i,j]) + part_{t-1}[b,i])
storing the full partition history. Host recomputes the (tiny) argmax
backpointers lazily along the decoded path and runs the sequential
backtrack, bit-exactly matching the float32 arithmetic of the reference.

Layout per core (16 batches): 8 chains, chain c covers local batches
{c, 8+c}. SBUF tiles are [128 partitions = (b2, j), free = i] where
b2 in {0,1} selects the batch of the chain and j is the current tag.

Chains are split into 2 groups of 4 so PE/DVE/ACT work of the two
groups overlaps. Per step t, per group g:
  - ACT copies base[t,g] (= feats+trans, prebuilt in bulk on GPSIMD)
    into a full-bank PSUM tile (has_written bits pre-primed once, so
    subsequent PE matmuls accumulate instead of overwrite).
  - 4 fp32 matmuls (one per chain) accumulate the broadcast previous
    part vector on top: cur = base + part_rep (exact reference order).
  - DVE tensor_reduce(max) writes part_t directly into part_hist.
  - PE transpose [128,4]->[4,128] + 2 ACT copies build the next step's
    moving operand pts2 [8=(b2,cl), 64=i].
"""

import sys
import os

sys.path.insert(0, "/opt/trn_rl_repo")

import numpy as np

import concourse.bass as bass
import concourse.mybir as mybir
import concourse.tile as tile
from concourse.vector_clock import ScopedClock

B, S, T = 128, 512, 64
START_TAG, STOP_TAG = T - 2, T - 1
N_CORES = 8
BPC = B // N_CORES          # batches per core = 16
NCH = BPC // 2              # chains per core = 8
NG = 4                      # chain groups (pipelined)
GCH = NCH // NG             # chains per group = 2
TCHUNK = 16                 # time steps per bulk base-build chunk
NEG_INIT = -3.4028234e38

_F32 = mybir.dt.float32


def _patch_tile_drain():
    """walrus in this toolchain rejects >1-2 sem waits on one CTRL
    instruction; split the TileContext tail-drain waits one-per-nop."""

    def _patched(self, tick_clock, wait_clock):
        carrier = self.nc.sync.nop()
        wait_clock.add_sem_waits(
            carrier.ins, ScopedClock({None: tick_clock.global_clock})
        )
        si = carrier.ins.sync_info
        waits = list(si.on_wait) if si and si.on_wait else []
        upds = list(si.on_update) if si and si.on_update else []
        if len(waits) > 1:
            carrier.ins.sync_info = mybir.SyncInfo(on_wait=[waits[0]], on_update=upds)
            for w in waits[1:]:
                n = self.nc.sync.nop()
                n.ins.sync_info = mybir.SyncInfo(on_wait=[w], on_update=[])
        self.nc.sync.drain()
        self.nc.all_engine_barrier()
        assert self.sems is not None
        popped = self.nc._tile_sem_poison_stack.pop()
        assert popped is self._sem_poison
        # Clear in chunks: large EVENT_SEMAPHORE_RANGE_CLEAR ranges hit
        # "ISA wrong length" in this walrus.
        self.nc.all_engine_barrier()

    tile.TileContext._drain_and_barrier = _patched

    # Same walrus limit applies to every instruction: compute/DMA structs
    # take only 1 sem wait, CTRL (nop/drain) takes 2. Split excess waits
    # onto preceding same-engine nop carriers.
    orig_add = tile.TileContext._add_instruction

    def _add_split(self, inst):
        si = getattr(inst, "sync_info", None)
        waits = list(si.on_wait) if si and si.on_wait else []
        lim = 1
        if len(waits) > lim:
            head, rest = waits[:lim], waits[lim:]
            for w in rest:
                carrier = mybir.InstNoOp(
                    name=self.nc.get_next_instruction_name(),
                    sync_info=mybir.SyncInfo(on_wait=[w], on_update=[]),
                    bass_nofuse=True,
                    engine=inst.engine,
                )
                orig_add(self, carrier)
            inst.sync_info = mybir.SyncInfo(
                on_wait=head, on_update=list(si.on_update or [])
            )
        orig_add(self, inst)

    tile.TileContext._add_instruction = _add_split


_patch_tile_drain()


def build_forward_kernel():
    """One NeuronCore's forward-scan bass module."""
    nc = bass.Bass()
    # featsT[b2*64 + j, s, c] = feats[b2*8 + c, s, j]  (host pre-transposed)
    featsT = nc.declare_dram_parameter("featsT", [128, S, NCH], _F32, isOutput=False)
    transT = nc.declare_dram_parameter("transT", [T, T], _F32, isOutput=False)
    ident = nc.declare_dram_parameter("ident", [128, 128], _F32, isOutput=False)
    parts = nc.declare_dram_parameter("parts", [128, S * NCH], _F32, isOutput=True)

    nchunks = S // TCHUNK

    with tile.TileContext(nc) as tc:
        from contextlib import ExitStack as _ES

        with _ES() as _es:
            # PSUM cur pools first: bank-aligned tiles. Each pool serves a
            # PAIR of groups with a two-bank tile (group-half per bank), so
            # one DVE reduce covers both groups. 2 pools x 2 bufs x 2 banks
            # = all 8 banks.
            curp = [
                _es.enter_context(
                    tc.tile_pool(name=f"cur{q}", bufs=2, space="PSUM")
                )
                for q in range(NG // 2)
            ]
            constp = _es.enter_context(tc.tile_pool(name="const", bufs=1))
            histp = _es.enter_context(tc.tile_pool(name="hist", bufs=1))
            ftp = _es.enter_context(tc.tile_pool(name="ft", bufs=1))
            basep = _es.enter_context(tc.tile_pool(name="base", bufs=2))

            # --- constants ---
            transRep = constp.tile([128, T], _F32, tag="transRep")
            # rows (b2, j) <- transT[j, :]  (= trans[:, j] along free i)
            nc.sync.dma_start(transRep[0:64, :], transT[:, :])
            nc.sync.dma_start(transRep[64:128, :], transT[:, :])
            identity = constp.tile([128, 128], _F32, tag="identity")
            nc.sync.dma_start(identity[:], ident[:])
            zeros_sb = constp.tile([1, 512], _F32, tag="zeros")
            nc.scalar.memzero(zeros_sb[:])

            # --- state tiles ---
            part_hist = histp.tile([128, S * NCH], _F32, tag="part_hist")

            # all of featsT in SBUF: [128=(b2,j), s, c]
            ft_all = ftp.tile([128, S, NCH], _F32, tag="ft")
            nc.sync.dma_start(ft_all[:], featsT[:])

            base_tiles = []

            def build_base(n):
                # gpsimd TT ISA encodes at most 3 AP dims: use (t,c)-merged views
                ft = ft_all[:, n * TCHUNK:(n + 1) * TCHUNK, :].rearrange(
                    "p s c -> p (s c)"
                )
                bt = basep.tile([128, TCHUNK * NCH, T], _F32, tag="base")
                in0 = ft.unsqueeze(2).broadcast_to([128, TCHUNK * NCH, T])
                in1 = (
                    transRep[:]
                    .unsqueeze(1)
                    .broadcast_to([128, TCHUNK * NCH, T])
                )
                # base = f + trans  (reference order: feats + transitions)
                nc.gpsimd.tensor_tensor(bt[:], in0, in1, mybir.AluOpType.add)
                return bt

            # prefetch first two chunks
            base_tiles.append(build_base(0))
            base_tiles.append(build_base(1))

            # --- t = 0: part0[:, c] = fT0[:, 0, c] + trans[START, j] ---
            nc.vector.tensor_scalar_add(
                part_hist[:, 0:NCH],
                ft_all[:, 0, :],
                transRep[:, START_TAG:START_TAG + 1],
            )

            bt4 = [b.rearrange("p (s c) i -> p s c i", c=NCH) for b in base_tiles]
            cur_tiles = [None] * NG

            for t in range(1, S):
                n, trel = divmod(t, TCHUNK)
                if trel == 0 and n >= 1 and n + 1 < nchunks:
                    base_tiles.append(build_base(n + 1))
                    bt4.append(base_tiles[-1].rearrange("p (s c) i -> p s c i", c=NCH))

                # Each two-bank cur tile holds a PAIR of groups (one bank
                # each), and each bank holds TWO consecutive steps (halving
                # both the ACT base-copy and the DVE reduce instruction
                # counts). Step pairs are (even t, odd t+1), never crossing
                # a TCHUNK.
                fresh = (t == 1) or (t % 2 == 0)
                half = 0 if fresh else 1

                for q in range(NG // 2):
                    if fresh:
                        curt = curp[q].tile([128, 1024], _F32, tag=f"cur{q}")
                        cur_tiles[q] = curt
                        if t <= 2:
                            # prime has_written bits of both banks once: a
                            # start=True matmul clears+sets a whole bank so
                            # later start=False matmuls accumulate onto ACT-
                            # written base values.
                            for bank in range(2):
                                nc.tensor.matmul(
                                    curt[:, bank * 512:(bank + 1) * 512],
                                    zeros_sb[:, 0:128],
                                    zeros_sb[:],
                                    start=True,
                                    stop=True,
                                )
                        nsteps = 1 if t == 1 else 2
                        # copy base for both groups of the pair: free dims
                        # (grp-bank, step, chain*i) to stay within AP limits
                        nc.scalar.copy(
                            curt[:, 0:1024].rearrange(
                                "p (G h f) -> p G h f", G=2, h=4
                            )[:, :, 0:nsteps, :],
                            bt4[n][:, trel:trel + nsteps,
                                   q * 2 * GCH:(q + 1) * 2 * GCH, :].rearrange(
                                "p s (G c) i -> p G s (c i)", G=2
                            ),
                        )
                    else:
                        curt = cur_tiles[q]
                    for gl in range(2):          # group-within-pair
                        g = q * 2 + gl
                        fbase = gl * 512 + half * GCH * T
                        for cl in range(GCH):
                            col = (t - 1) * NCH + g * GCH + cl
                            for b2 in range(2):
                                # b2=0 and b2=1 land on disjoint diagonal
                                # quadrants of the PE array (rows/cols 0-63
                                # vs 64-127), so tile_position lets them
                                # execute concurrently.
                                nc.tensor.matmul(
                                    curt[b2 * 64:(b2 + 1) * 64,
                                         fbase + cl * T:fbase + (cl + 1) * T],
                                    part_hist[
                                        b2 * 64:(b2 + 1) * 64, col:col + 1
                                    ].broadcast_to([64, T]),
                                    identity[b2 * 64:(b2 + 1) * 64,
                                             b2 * 64:(b2 + 1) * 64],
                                    start=False,
                                    stop=True,
                                    skip_group_check=True,
                                    tile_position=(b2 * 64, b2 * 64),
                                )

                # phase 2: one max-reduce per group-pair -> part_hist (DVE)
                for q in range(NG // 2):
                    curt = cur_tiles[q]
                    in4 = curt[:, 0:1024].rearrange(
                        "p (G h c i) -> p G h c i", G=2, h=4, c=GCH
                    )[:, :, half, :, :]
                    nc.vector.tensor_reduce(
                        part_hist[:, t * NCH + q * 2 * GCH:
                                  t * NCH + (q + 1) * 2 * GCH],
                        in4,
                        axis=mybir.AxisListType.X,
                        op=mybir.AluOpType.max,
                    )

            nc.sync.dma_start(parts[:], part_hist[:])

    return nc


_FWD_CACHE = {}
LAST_EXEC_NS = None


def _forward_on_device(feats_np, trans_np):
    """Run the forward scan on 8 cores. Returns part_hist [S, B, T] f32."""
    global LAST_EXEC_NS
    from concourse.bass_utils import run_bass_kernel_spmd

    if "nc" not in _FWD_CACHE:
        _FWD_CACHE["nc"] = build_forward_kernel()
    nc = _FWD_CACHE["nc"]

    transT = np.ascontiguousarray(trans_np.T)
    ident = np.eye(128, dtype=np.float32)
    in_maps = []
    for k in range(N_CORES):
        shard = feats_np[k * BPC:(k + 1) * BPC]          # (16, S, T)
        # featsT[b2*64 + j, s, c] = shard[b2*8 + c, s, j]
        ft = np.ascontiguousarray(
            shard.reshape(2, NCH, S, T).transpose(0, 3, 2, 1).reshape(128, S, NCH)
        )
        in_maps.append(
            {"featsT": ft, "transT": transT, "ident": ident}
        )

    trace = bool(os.environ.get("CRF_TRACE"))
    res = run_bass_kernel_spmd(
        nc, in_maps, list(range(N_CORES)), trace=trace
    )
    if res.exec_time_ns is not None:
        LAST_EXEC_NS = res.exec_time_ns

    part = np.empty((S, B, T), dtype=np.float32)
    for k in range(N_CORES):
        p = res.results[k]["parts"].reshape(128, S, NCH)  # [(b2,j), t, c]
        p = p.reshape(2, T, S, NCH)                       # [b2, j, t, c]
        # local batch = b2*8 + c
        part[:, k * BPC:(k + 1) * BPC, :] = (
            p.transpose(2, 0, 3, 1).reshape(S, BPC, T)
        )
    return part


def _host_backtrack(part, feats, mask, trans):
    """Backpointer recompute + backtrack, bit-exact vs the jax reference."""
    f32 = np.float32
    lengths = mask.astype(np.int64).sum(axis=1)          # (B,)
    last_pos = lengths - 1
    bidx = np.arange(B)

    last_partition = part[last_pos, bidx, :]             # (B, T)
    # pointer0 = argmax_i(last_partition[b,i] + trans[i, STOP])
    last_vals = last_partition + trans[:, STOP_TAG][None, :].astype(f32)
    pointer0 = np.argmax(last_vals, axis=1).astype(np.int32)

    decode = np.zeros((S, B), dtype=np.int32)
    decode[S - 1] = pointer0
    ptr = pointer0
    trans_T = np.ascontiguousarray(trans.T)              # trans_T[j, i] = trans[i, j]
    for t in range(S - 2, -1, -1):
        jstar = ptr                                       # decode[t+1]
        fcol = feats[bidx, t + 1, jstar].astype(f32)      # (B,)
        cur = (fcol[:, None] + trans_T[jstar]) + part[t]  # (B, T) f32
        bp_val = np.argmax(cur, axis=1).astype(np.int32)
        new_ptr = np.where(
            t == last_pos, pointer0,
            np.where(t >= lengths, 0, bp_val)
        ).astype(np.int32)
        decode[t] = new_ptr
        ptr = new_ptr
    return decode.T                                       # (B, S)


def kernel(feats, mask, tags, transitions):
    feats = np.asarray(feats, dtype=np.float32)
    mask = np.asarray(mask)
    trans = np.asarray(transitions, dtype=np.float32)
    part = _forward_on_device(feats, trans)
    return _host_backtrack(part, feats, mask, trans)


# revision 34
# speedup vs baseline: 1.1253x; 1.1253x over previous
"""Viterbi CRF decode kernel for Trainium2, data-parallel over batch on 8 cores.

Device computes the forward Viterbi max-plus scan (the O(S*B*T^2) bulk):
  part_t[b, j] = max_i((feats[t,b,j] + trans[i,j]) + part_{t-1}[b,i])
storing the full partition history. Host recomputes the (tiny) argmax
backpointers lazily along the decoded path and runs the sequential
backtrack, bit-exactly matching the float32 arithmetic of the reference.

Layout per core (16 batches): 8 chains, chain c covers local batches
{c, 8+c}. SBUF tiles are [128 partitions = (b2, j), free = i] where
b2 in {0,1} selects the batch of the chain and j is the current tag.

Chains are split into 2 groups of 4 so PE/DVE/ACT work of the two
groups overlaps. Per step t, per group g:
  - ACT copies base[t,g] (= feats+trans, prebuilt in bulk on GPSIMD)
    into a full-bank PSUM tile (has_written bits pre-primed once, so
    subsequent PE matmuls accumulate instead of overwrite).
  - 4 fp32 matmuls (one per chain) accumulate the broadcast previous
    part vector on top: cur = base + part_rep (exact reference order).
  - DVE tensor_reduce(max) writes part_t directly into part_hist.
  - PE transpose [128,4]->[4,128] + 2 ACT copies build the next step's
    moving operand pts2 [8=(b2,cl), 64=i].
"""

import sys
import os

sys.path.insert(0, "/opt/trn_rl_repo")

import numpy as np

import concourse.bass as bass
import concourse.mybir as mybir
import concourse.tile as tile
from concourse.vector_clock import ScopedClock

B, S, T = 128, 512, 64
START_TAG, STOP_TAG = T - 2, T - 1
N_CORES = 8
BPC = B // N_CORES          # batches per core = 16
NCH = BPC // 2              # chains per core = 8
NG = 4                      # chain groups (pipelined)
GCH = NCH // NG             # chains per group = 2
TCHUNK = 16                 # time steps per bulk base-build chunk
NEG_INIT = -3.4028234e38

_F32 = mybir.dt.float32


def _patch_tile_drain():
    """walrus in this toolchain rejects >1-2 sem waits on one CTRL
    instruction; split the TileContext tail-drain waits one-per-nop."""

    def _patched(self, tick_clock, wait_clock):
        carrier = self.nc.sync.nop()
        wait_clock.add_sem_waits(
            carrier.ins, ScopedClock({None: tick_clock.global_clock})
        )
        si = carrier.ins.sync_info
        waits = list(si.on_wait) if si and si.on_wait else []
        upds = list(si.on_update) if si and si.on_update else []
        if len(waits) > 1:
            carrier.ins.sync_info = mybir.SyncInfo(on_wait=[waits[0]], on_update=upds)
            for w in waits[1:]:
                n = self.nc.sync.nop()
                n.ins.sync_info = mybir.SyncInfo(on_wait=[w], on_update=[])
        self.nc.sync.drain()
        self.nc.all_engine_barrier()
        assert self.sems is not None
        popped = self.nc._tile_sem_poison_stack.pop()
        assert popped is self._sem_poison
        # Clear in chunks: large EVENT_SEMAPHORE_RANGE_CLEAR ranges hit
        # "ISA wrong length" in this walrus.
        self.nc.all_engine_barrier()

    tile.TileContext._drain_and_barrier = _patched

    # Same walrus limit applies to every instruction: compute/DMA structs
    # take only 1 sem wait, CTRL (nop/drain) takes 2. Split excess waits
    # onto preceding same-engine nop carriers.
    orig_add = tile.TileContext._add_instruction

    def _add_split(self, inst):
        si = getattr(inst, "sync_info", None)
        waits = list(si.on_wait) if si and si.on_wait else []
        lim = 1
        if len(waits) > lim:
            head, rest = waits[:lim], waits[lim:]
            for w in rest:
                carrier = mybir.InstNoOp(
                    name=self.nc.get_next_instruction_name(),
                    sync_info=mybir.SyncInfo(on_wait=[w], on_update=[]),
                    bass_nofuse=True,
                    engine=inst.engine,
                )
                orig_add(self, carrier)
            inst.sync_info = mybir.SyncInfo(
                on_wait=head, on_update=list(si.on_update or [])
            )
        orig_add(self, inst)

    tile.TileContext._add_instruction = _add_split


_patch_tile_drain()


def build_forward_kernel():
    """One NeuronCore's forward-scan bass module."""
    nc = bass.Bass()
    # featsT[b2*64 + j, s, c] = feats[b2*8 + c, s, j]  (host pre-transposed)
    featsT = nc.declare_dram_parameter("featsT", [128, S, NCH], _F32, isOutput=False)
    transT = nc.declare_dram_parameter("transT", [T, T], _F32, isOutput=False)
    ident = nc.declare_dram_parameter("ident", [128, 128], _F32, isOutput=False)
    parts = nc.declare_dram_parameter("parts", [128, S * NCH], _F32, isOutput=True)

    nchunks = S // TCHUNK

    with tile.TileContext(nc) as tc:
        from contextlib import ExitStack as _ES

        with _ES() as _es:
            # PSUM cur pools first: full-bank tiles stay bank-aligned.
            # 4 groups x 2 bufs = all 8 banks.
            curp = [
                _es.enter_context(
                    tc.tile_pool(name=f"cur{g}", bufs=2, space="PSUM")
                )
                for g in range(NG)
            ]
            constp = _es.enter_context(tc.tile_pool(name="const", bufs=1))
            histp = _es.enter_context(tc.tile_pool(name="hist", bufs=1))
            ftp = _es.enter_context(tc.tile_pool(name="ft", bufs=1))
            basep = _es.enter_context(tc.tile_pool(name="base", bufs=2))

            # --- constants ---
            transRep = constp.tile([128, T], _F32, tag="transRep")
            # rows (b2, j) <- transT[j, :]  (= trans[:, j] along free i)
            nc.sync.dma_start(transRep[0:64, :], transT[:, :])
            nc.sync.dma_start(transRep[64:128, :], transT[:, :])
            identity = constp.tile([128, 128], _F32, tag="identity")
            nc.sync.dma_start(identity[:], ident[:])
            zeros_sb = constp.tile([1, 512], _F32, tag="zeros")
            nc.scalar.memzero(zeros_sb[:])

            # --- state tiles ---
            part_hist = histp.tile([128, S * NCH], _F32, tag="part_hist")

            # all of featsT in SBUF: [128=(b2,j), s, c]
            ft_all = ftp.tile([128, S, NCH], _F32, tag="ft")
            nc.sync.dma_start(ft_all[:], featsT[:])

            base_tiles = []

            def build_base(n):
                # gpsimd TT ISA encodes at most 3 AP dims: use (t,c)-merged views
                ft = ft_all[:, n * TCHUNK:(n + 1) * TCHUNK, :].rearrange(
                    "p s c -> p (s c)"
                )
                bt = basep.tile([128, TCHUNK * NCH, T], _F32, tag="base")
                in0 = ft.unsqueeze(2).broadcast_to([128, TCHUNK * NCH, T])
                in1 = (
                    transRep[:]
                    .unsqueeze(1)
                    .broadcast_to([128, TCHUNK * NCH, T])
                )
                # base = f + trans  (reference order: feats + transitions)
                nc.gpsimd.tensor_tensor(bt[:], in0, in1, mybir.AluOpType.add)
                return bt

            # prefetch first two chunks
            base_tiles.append(build_base(0))
            base_tiles.append(build_base(1))

            # --- t = 0: part0[:, c] = fT0[:, 0, c] + trans[START, j] ---
            nc.vector.tensor_scalar_add(
                part_hist[:, 0:NCH],
                ft_all[:, 0, :],
                transRep[:, START_TAG:START_TAG + 1],
            )

            bt4 = [b.rearrange("p (s c) i -> p s c i", c=NCH) for b in base_tiles]
            cur_tiles = [None] * NG

            for t in range(1, S):
                n, trel = divmod(t, TCHUNK)
                if trel == 0 and n >= 1 and n + 1 < nchunks:
                    base_tiles.append(build_base(n + 1))
                    bt4.append(base_tiles[-1].rearrange("p (s c) i -> p s c i", c=NCH))

                # Each full-bank cur tile holds TWO consecutive steps of a
                # group (halving the ACT base-copy instruction count). Pairs
                # are (even t, odd t+1), so they never cross a TCHUNK.
                fresh = (t == 1) or (t % 2 == 0)
                half = 0 if fresh else 1

                cur_now = []
                for g in range(NG):
                    if fresh:
                        curt = curp[g].tile([128, 512], _F32, tag=f"cur{g}")
                        cur_tiles[g] = curt
                        if t <= 2:
                            # prime has_written bits of this bank once: a
                            # start=True matmul clears+sets the whole bank so
                            # later start=False matmuls accumulate onto ACT-
                            # written base values.
                            nc.tensor.matmul(
                                curt[:],
                                zeros_sb[:, 0:128],
                                zeros_sb[:],
                                start=True,
                                stop=True,
                            )
                        nsteps = 1 if t == 1 or t == S - 1 else 2
                        nc.scalar.copy(
                            curt[:, 0:nsteps * GCH * T].rearrange(
                                "p (s c i) -> p s c i", s=nsteps, c=GCH
                            ),
                            bt4[n][:, trel:trel + nsteps,
                                   g * GCH:(g + 1) * GCH, :],
                        )
                    else:
                        curt = cur_tiles[g]
                    fbase = half * GCH * T
                    cur3 = curt[:, fbase:fbase + GCH * T].rearrange(
                        "p (c i) -> p c i", c=GCH
                    )
                    for cl in range(GCH):
                        col = (t - 1) * NCH + g * GCH + cl
                        for b2 in range(2):
                            # b2=0 and b2=1 land on disjoint diagonal
                            # quadrants of the PE array (rows/cols 0-63 vs
                            # 64-127), so tile_position lets them execute
                            # concurrently.
                            nc.tensor.matmul(
                                curt[b2 * 64:(b2 + 1) * 64,
                                     fbase + cl * T:fbase + (cl + 1) * T],
                                part_hist[
                                    b2 * 64:(b2 + 1) * 64, col:col + 1
                                ].broadcast_to([64, T]),
                                identity[b2 * 64:(b2 + 1) * 64,
                                         b2 * 64:(b2 + 1) * 64],
                                start=False,
                                stop=True,
                                skip_group_check=True,
                                tile_position=(b2 * 64, b2 * 64),
                            )
                    cur_now.append(cur3)

                # phase 2: max-reduce -> part_hist (DVE)
                for g in range(NG):
                    nc.vector.tensor_reduce(
                        part_hist[:, t * NCH + g * GCH:t * NCH + (g + 1) * GCH],
                        cur_now[g],
                        axis=mybir.AxisListType.X,
                        op=mybir.AluOpType.max,
                    )

            nc.sync.dma_start(parts[:], part_hist[:])

    return nc


_FWD_CACHE = {}
LAST_EXEC_NS = None


def _forward_on_device(feats_np, trans_np):
    """Run the forward scan on 8 cores. Returns part_hist [S, B, T] f32."""
    global LAST_EXEC_NS
    from concourse.bass_utils import run_bass_kernel_spmd

    if "nc" not in _FWD_CACHE:
        _FWD_CACHE["nc"] = build_forward_kernel()
    nc = _FWD_CACHE["nc"]

    transT = np.ascontiguousarray(trans_np.T)
    ident = np.eye(128, dtype=np.float32)
    in_maps = []
    for k in range(N_CORES):
        shard = feats_np[k * BPC:(k + 1) * BPC]          # (16, S, T)
        # featsT[b2*64 + j, s, c] = shard[b2*8 + c, s, j]
        ft = np.ascontiguousarray(
            shard.reshape(2, NCH, S, T).transpose(0, 3, 2, 1).reshape(128, S, NCH)
        )
        in_maps.append(
            {"featsT": ft, "transT": transT, "ident": ident}
        )

    trace = bool(os.environ.get("CRF_TRACE"))
    res = run_bass_kernel_spmd(
        nc, in_maps, list(range(N_CORES)), trace=trace
    )
    if res.exec_time_ns is not None:
        LAST_EXEC_NS = res.exec_time_ns

    part = np.empty((S, B, T), dtype=np.float32)
    for k in range(N_CORES):
        p = res.results[k]["parts"].reshape(128, S, NCH)  # [(b2,j), t, c]
        p = p.reshape(2, T, S, NCH)                       # [b2, j, t, c]
        # local batch = b2*8 + c
        part[:, k * BPC:(k + 1) * BPC, :] = (
            p.transpose(2, 0, 3, 1).reshape(S, BPC, T)
        )
    return part


def _host_backtrack(part, feats, mask, trans):
    """Backpointer recompute + backtrack, bit-exact vs the jax reference."""
    f32 = np.float32
    lengths = mask.astype(np.int64).sum(axis=1)          # (B,)
    last_pos = lengths - 1
    bidx = np.arange(B)

    last_partition = part[last_pos, bidx, :]             # (B, T)
    # pointer0 = argmax_i(last_partition[b,i] + trans[i, STOP])
    last_vals = last_partition + trans[:, STOP_TAG][None, :].astype(f32)
    pointer0 = np.argmax(last_vals, axis=1).astype(np.int32)

    decode = np.zeros((S, B), dtype=np.int32)
    decode[S - 1] = pointer0
    ptr = pointer0
    trans_T = np.ascontiguousarray(trans.T)              # trans_T[j, i] = trans[i, j]
    for t in range(S - 2, -1, -1):
        jstar = ptr                                       # decode[t+1]
        fcol = feats[bidx, t + 1, jstar].astype(f32)      # (B,)
        cur = (fcol[:, None] + trans_T[jstar]) + part[t]  # (B, T) f32
        bp_val = np.argmax(cur, axis=1).astype(np.int32)
        new_ptr = np.where(
            t == last_pos, pointer0,
            np.where(t >= lengths, 0, bp_val)
        ).astype(np.int32)
        decode[t] = new_ptr
        ptr = new_ptr
    return decode.T                                       # (B, S)


def kernel(feats, mask, tags, transitions):
    feats = np.asarray(feats, dtype=np.float32)
    mask = np.asarray(mask)
    trans = np.asarray(transitions, dtype=np.float32)
    part = _forward_on_device(feats, trans)
    return _host_backtrack(part, feats, mask, trans)
